# revision 1
# baseline (speedup 1.0000x reference)
"""Trainium2 Bass kernel for nn_CausalNeuralHawkesMasked (CTLSTM / Neural Hawkes scan).

Strategy (8-core pure data parallel over batch):
  - B=512 batches sharded 64/core; each core runs the full S-1=2047 step recurrence.
  - Layout: batch on partitions (64), gates/hidden on the free dim.
  - Per step: PE accumulates g = onehot(type).T @ G + H2T.T @ Wh into PSUM [64, 224]
    (G = emb@Wx + b, columns permuted+scaled so ONE tanh covers all sigmoid gates
     in tanh space; d-gate column pre-scaled by beta).
  - ACT uses only the `exp_and_others` table set: tanh, square, exp.
    softplus(t)*10 is evaluated as an even polynomial in t (|t| <~ 0.2, Taylor
    error < 1e-7), sigmoid gates are tanh-encoded, o/h/c outputs get cheap exact
    host-side affine fixups.
  - Carry kept 2x-scaled so every sigmoid fixup folds into fused
    scalar_tensor_tensor ops with zero extra instructions.

Device outputs (per core) OUT[5, 64, S-1, 32]:
  j=0: 2*h, j=1: 2*c_new, j=2: 2*cbar_new, j=3: delta (exact), j=4: tanh-space o.
Host: halve/affine-fix + transpose + concat cores.
"""

import os
import sys

import numpy as np

if "/opt/trn_rl_repo" not in sys.path:
    sys.path.insert(0, "/opt/trn_rl_repo")

N_TYPES = 20
NT = N_TYPES + 1  # embedding rows
HID = 32
BETA = 0.1
B_FULL = 512
S_FULL = 2048
N_CORES = 8
B_CORE = B_FULL // N_CORES  # 64

# softplus(t)*10 Taylor coefficients (even series + linear term)
C0 = 10.0 * float(np.log(2.0))
C1 = 5.0
C2 = 10.0 / 8.0
C4 = -10.0 / 192.0

# gate order used on-device (original order in W: [i, f, z, o, ib, fb, d])
# new order: [fb, f, ib, i, o, z, d]
#   - V1a pairs (fb,f) against carry [cbar2|ct2]
#   - Pz pairs (ib,i) against [z|z]
_PERM = [5, 1, 4, 0, 3, 2, 6]
_COL_SCALE = [0.5, 0.5, 0.5, 0.5, 0.5, 1.0, BETA]


def _host_params(emb, W, b):
    """Return (G_tilde [21,224], Wh_eff [32,224]) with permuted+scaled columns."""
    emb = np.asarray(emb, np.float32)
    W = np.asarray(W, np.float32)
    b = np.asarray(b, np.float32)
    G = emb @ W[:HID] + b  # [21, 224]
    Wh = W[HID:]  # [32, 224]
    Gp = np.empty_like(G)
    Whp = np.empty_like(Wh)
    for k, (p, sc) in enumerate(zip(_PERM, _COL_SCALE)):
        Gp[:, k * HID : (k + 1) * HID] = G[:, p * HID : (p + 1) * HID] * sc
        Whp[:, k * HID : (k + 1) * HID] = Wh[:, p * HID : (p + 1) * HID] * sc
    # h enters the matmul as H2 = 2h, so halve Wh once more
    Whp *= 0.5
    return Gp, Whp


def build_nc(nsteps, toh=128, blk=32, pre=4):
    """Build the Bass program for one core (SPMD across 8). Returns (nc, names)."""
    import concourse.bacc as bacc
    import concourse.bass as bass
    import concourse.tile as tile
    from concourse import mybir

    f32 = mybir.dt.float32
    AF = mybir.ActivationFunctionType
    OP = mybir.AluOpType

    assert nsteps % 1 == 0
    nblocks = (nsteps + blk - 1) // blk
    nchunks = (nsteps + toh - 1) // toh

    nc = bacc.Bacc(None, target_bir_lowering=False)
    names = {}
    with tile.TileContext(nc) as tc:
        from contextlib import ExitStack

        with ExitStack() as ctx:
            dram = ctx.enter_context(tc.tile_pool(name="dram", bufs=1, space="DRAM"))
            oht_d = dram.tile([NT, nsteps * B_CORE], f32, kind="ExternalInput")
            ndt_d = dram.tile([B_CORE, S_FULL], f32, kind="ExternalInput")
            g_d = dram.tile([NT, 7 * HID], f32, kind="ExternalInput")
            wh_d = dram.tile([HID, 7 * HID], f32, kind="ExternalInput")
            out_d = dram.tile([5, B_CORE, nsteps, HID], f32, kind="ExternalOutput")
            names.update(
                oht=oht_d.name, ndt=ndt_d.name, g=g_d.name, wh=wh_d.name, out=out_d.name
            )

            singles = ctx.enter_context(tc.tile_pool(name="singles", bufs=1))
            ohp = ctx.enter_context(tc.tile_pool(name="ohp", bufs=2))
            psum = ctx.enter_context(
                tc.tile_pool(name="psum", bufs=pre + 2, space="PSUM")
            )
            t1p = ctx.enter_context(tc.tile_pool(name="t1p", bufs=2))
            combp = ctx.enter_context(tc.tile_pool(name="combp", bufs=2))
            hp = ctx.enter_context(tc.tile_pool(name="hp", bufs=2))
            dlp = ctx.enter_context(tc.tile_pool(name="dlp", bufs=2))
            scr = ctx.enter_context(tc.tile_pool(name="scr", bufs=4))
            htp = ctx.enter_context(tc.tile_pool(name="htp", bufs=3))

            # resident tensors
            g_sb = singles.tile([NT, 7 * HID], f32)
            wh_sb = singles.tile([HID, 7 * HID], f32)
            ndt_sb = singles.tile([B_CORE, S_FULL], f32)
            ht0 = singles.tile([HID, B_CORE], f32)
            cc0 = singles.tile([B_CORE, 2 * HID], f32)
            nc.sync.dma_start(out=g_sb, in_=g_d[:])
            nc.sync.dma_start(out=wh_sb, in_=wh_d[:])
            nc.sync.dma_start(out=ndt_sb, in_=ndt_d[:])
            nc.vector.memset(ht0, 0.0)
            nc.vector.memset(cc0, 0.0)

            # one-hot chunk tiles (list so prefetch can refer to them)
            oh_tiles = {}

            def load_chunk(c):
                if c >= nchunks or c in oh_tiles:
                    return
                cs = min(toh, nsteps - c * toh)
                t = ohp.tile([NT, toh * B_CORE], f32, tag="ohchunk")
                nc.sync.dma_start(
                    out=t[:, : cs * B_CORE],
                    in_=oht_d[:, c * toh * B_CORE : (c * toh + cs) * B_CORE],
                )
                oh_tiles[c] = t

            psum_tiles = {}

            def emit_xmm(s):
                if s >= nsteps or s in psum_tiles:
                    return
                c = s // toh
                # prefetch next chunk at mid-chunk
                if (s % toh) == toh // 2:
                    load_chunk(c + 1)
                pt = psum.tile([B_CORE, 7 * HID], f32, tag="gates")
                lhs = oh_tiles[c][:, (s - c * toh) * B_CORE : (s - c * toh + 1) * B_CORE]
                nc.tensor.matmul(pt, lhs, g_sb, start=True, stop=False)
                psum_tiles[s] = pt

            load_chunk(0)
            for s in range(pre):
                emit_xmm(s)

            prev_carry = cc0  # [64, 64] = [cbar2 | ct2]
            prev_ht = ht0  # [32, 64] = (2h)^T

            for bi in range(nblocks):
                t0 = bi * blk
                bs = min(blk, nsteps - t0)
                T1 = t1p.tile([B_CORE, blk, 6 * HID], f32, tag="t1")
                COMB = combp.tile([B_CORE, blk, 3 * HID], f32, tag="comb")
                Hb = hp.tile([B_CORE, blk, HID], f32, tag="hb")
                DL = dlp.tile([B_CORE, blk, HID], f32, tag="dl")

                for j in range(bs):
                    s = t0 + j
                    emit_xmm(s + pre)
                    pt = psum_tiles.pop(s)
                    # h-part accumulate
                    nc.tensor.matmul(pt, prev_ht, wh_sb, start=False, stop=True)

                    t1s = T1[:, j, :]
                    # gates tanh: [fb~, f~, ib~, i~, o~, z~] <- cols 0:192
                    nc.scalar.activation(t1s, pt[:, 0 : 6 * HID], AF.Tanh)
                    # d-path: t = pt[:, 192:224] (= beta*gd)
                    sq = scr.tile([B_CORE, HID], f32, tag="sq")
                    nc.scalar.activation(sq, pt[:, 6 * HID : 7 * HID], AF.Square)
                    u = scr.tile([B_CORE, HID], f32, tag="u")
                    nc.vector.tensor_scalar(u, sq, C4, C2, OP.mult, OP.add)
                    w = scr.tile([B_CORE, HID], f32, tag="w")
                    nc.vector.tensor_tensor(w, u, sq, OP.mult)
                    dls = DL[:, j, :]
                    nc.vector.affine_then_add(
                        dls, pt[:, 6 * HID : 7 * HID], w, scale=C1, bias=C0
                    )
                    # e = exp(-dt * delta)
                    e = scr.tile([B_CORE, HID], f32, tag="e")
                    nc.scalar.activation(
                        e, dls, AF.Exp, scale=ndt_sb[:, s + 1 : s + 2]
                    )
                    # Pa4 = (1 + [fb~|f~]) * [cbar2|ct2]
                    pa = scr.tile([B_CORE, 2 * HID], f32, tag="pa")
                    nc.vector.scalar_tensor_tensor(
                        pa, t1s[:, 0 : 2 * HID], 1.0, prev_carry, OP.add, OP.mult
                    )
                    # Pz2 = (1 + [ib~|i~]) * [z~|z~]
                    zt = t1s[:, 5 * HID : 6 * HID]
                    zz = bass.AP(
                        tensor=zt.tensor,
                        offset=zt.offset,
                        ap=[zt.ap[0], [0, 2], [1, HID]],
                    )
                    ii = t1s[:, 2 * HID : 4 * HID]
                    ii3 = bass.AP(
                        tensor=ii.tensor,
                        offset=ii.offset,
                        ap=[ii.ap[0], [HID, 2], [1, HID]],
                    )
                    pz = scr.tile([B_CORE, 2 * HID], f32, tag="pz")
                    pz3 = bass.AP(
                        tensor=pz.tensor,
                        offset=pz.offset,
                        ap=[pz.ap[0], [HID, 2], [1, HID]],
                    )
                    nc.vector.scalar_tensor_tensor(pz3, ii3, 1.0, zz, OP.add, OP.mult)
                    # COMB slot = [cbar2' | ct2 | c2']
                    combs = COMB[:, j, :]
                    cc_out = bass.AP(
                        tensor=combs.tensor,
                        offset=combs.offset,
                        ap=[combs.ap[0], [2 * HID, 2], [1, HID]],
                    )
                    pa3 = bass.AP(
                        tensor=pa.tensor,
                        offset=pa.offset,
                        ap=[pa.ap[0], [HID, 2], [1, HID]],
                    )
                    # [cbar2'|c2'] = 0.5*Pa4 + Pz2  (written to cols {0:32, 64:96})
                    nc.vector.scalar_tensor_tensor(
                        cc_out, pa3, 0.5, pz3, OP.mult, OP.add
                    )
                    # D2 = c2' - cbar2'
                    dv = scr.tile([B_CORE, HID], f32, tag="dv")
                    nc.vector.tensor_tensor(
                        dv, combs[:, 2 * HID : 3 * HID], combs[:, 0:HID], OP.subtract
                    )
                    de = scr.tile([B_CORE, HID], f32, tag="de")
                    nc.vector.tensor_tensor(de, dv, e, OP.mult)
                    # ct2 = cbar2' + D2*e -> col 32:64
                    nc.vector.tensor_tensor(
                        combs[:, HID : 2 * HID], combs[:, 0:HID], de, OP.add
                    )
                    th = scr.tile([B_CORE, HID], f32, tag="th")
                    nc.scalar.activation(
                        th, combs[:, HID : 2 * HID], AF.Tanh, scale=0.5
                    )
                    # H2 = (1 + o~) * th
                    hbs = Hb[:, j, :]
                    nc.vector.scalar_tensor_tensor(
                        hbs, t1s[:, 4 * HID : 5 * HID], 1.0, th, OP.add, OP.mult
                    )
                    # transpose H2 -> [32, 64]
                    ht = htp.tile([HID, B_CORE], f32, tag="ht")
                    nc.vector.transpose(ht[:, 0:HID], Hb[0:HID, j, :])
                    nc.vector.transpose(ht[:, HID:B_CORE], Hb[HID:B_CORE, j, :])

                    prev_carry = combs[:, 0 : 2 * HID]
                    prev_ht = ht

                # block DMAs
                nc.sync.dma_start(
                    out=out_d[0, :, t0 : t0 + bs, :], in_=Hb[:, :bs, :]
                )
                nc.sync.dma_start(
                    out=out_d[1, :, t0 : t0 + bs, :],
                    in_=COMB[:, :bs, 2 * HID : 3 * HID],
                )
                nc.sync.dma_start(
                    out=out_d[2, :, t0 : t0 + bs, :], in_=COMB[:, :bs, 0:HID]
                )
                nc.sync.dma_start(
                    out=out_d[3, :, t0 : t0 + bs, :], in_=DL[:, :bs, :]
                )
                nc.sync.dma_start(
                    out=out_d[4, :, t0 : t0 + bs, :],
                    in_=T1[:, :bs, 4 * HID : 5 * HID],
                )

    nc.compile()
    return nc, names


def _host_inputs(types, dtime, emb, W, b, nsteps):
    """Per-core input maps (list of dicts keyed later by names)."""
    types = np.asarray(types)
    dtime = np.asarray(dtime, np.float32)
    Gp, Whp = _host_params(emb, W, b)
    per_core = []
    for k in range(N_CORES):
        tc_ = np.asarray(types[k * B_CORE : (k + 1) * B_CORE, :nsteps])
        # one-hot transposed: [21, nsteps, 64] -> [21, nsteps*64]
        oh = np.zeros((NT, nsteps, B_CORE), np.float32)
        s_idx, b_idx = np.meshgrid(
            np.arange(nsteps), np.arange(B_CORE), indexing="ij"
        )
        oh[tc_.T, s_idx, b_idx] = 1.0
        ndt = -dtime[k * B_CORE : (k + 1) * B_CORE]  # [64, 2048]
        if ndt.shape[1] < S_FULL:
            pad = np.zeros((B_CORE, S_FULL - ndt.shape[1]), np.float32)
            ndt = np.concatenate([ndt, pad], 1)
        per_core.append(
            dict(
                oht=np.ascontiguousarray(oh.reshape(NT, nsteps * B_CORE)),
                ndt=np.ascontiguousarray(ndt),
                g=Gp,
                wh=Whp,
            )
        )
    return per_core


def _postprocess(raws, nsteps):
    """raws: list of OUT arrays [5, 64, nsteps, 32] per core -> 5 full outputs."""
    outs = []
    for j in range(5):
        full = np.empty((nsteps, B_FULL, HID), np.float32)
        for k in range(N_CORES):
            full[:, k * B_CORE : (k + 1) * B_CORE, :] = raws[k][j].transpose(1, 0, 2)
        outs.append(full)
    h2, c2, cb2, dl, ot = outs
    h = 0.5 * h2
    c = 0.5 * c2
    cb = 0.5 * cb2
    o = 0.5 * ot + 0.5
    return h, c, cb, dl, o


def kernel(types, dtime, emb, W, b, _trace=False, _nsteps=None):
    from concourse.bass_utils import run_bass_kernel_spmd

    nsteps = (S_FULL - 1) if _nsteps is None else _nsteps
    nc, names = build_nc(nsteps)
    per_core = _host_inputs(types, dtime, emb, W, b, nsteps)
    in_maps = [
        {names[k2]: v for k2, v in m.items()} for m in per_core
    ]
    res = run_bass_kernel_spmd(
        nc, in_maps, core_ids=list(range(N_CORES)), trace=_trace
    )
    raws = [res.results[i][names["out"]] for i in range(N_CORES)]
    out = _postprocess(raws, nsteps)
    if _trace:
        kernel._last_results = res
    return out
